# revision 2
# baseline (speedup 1.0000x reference)
"""H2GCN encoder on 8 Trainium2 NeuronCores.

Dest-sharded graph parallel: nodes split across 8 cores (12500 each,
padded 12544). Edges routed to the dest core, sorted by 256-wide dest
windows. Aggregation = one-hot matmul scatter: per 128-edge slot, gather
source rows (indirect DMA), build S[e,d]=(dest==d)*invdeg on DVE
(fused tensor_scalar vs iota), PSUM += Xg^T @ S (f32r full-rate matmul).
Feature-major throughout; two launches with host-side allgather of x1.
"""
import sys
sys.path.insert(0, '/opt/trn_rl_repo')
import numpy as np

import concourse.bass as bass
import concourse.bacc as bacc
import concourse.tile as tile
from concourse import mybir
from concourse import bass_utils

N = 100000
E = 1600000
D = 128
NC = 8
NLOC = 12500
WPAIR = 256
NWINP = 49
NLOC_PAD = NWINP * WPAIR     # 12544
NPAD = NC * NLOC_PAD         # 100352
P = 128

f32 = mybir.dt.float32
f32r = mybir.dt.float32r
i32 = mybir.dt.int32

_CACHE = {}


def _prep(edge_index, deg_inv):
    row = edge_index[0].astype(np.int64)
    col = edge_index[1].astype(np.int64)
    self_idx = np.arange(N, dtype=np.int64)
    row = np.concatenate([row, self_idx])
    col = np.concatenate([col, self_idx])

    core = row // NLOC
    dloc = row - core * NLOC
    win = dloc // WPAIR
    dstl = dloc - win * WPAIR

    cnt = np.zeros((NC, NWINP), np.int64)
    np.add.at(cnt, (core, win), 1)
    caps = ((cnt.max(axis=0) + P - 1) // P * P).astype(np.int64)
    S_per_w = caps // P
    S_total = int(S_per_w.sum())
    w_slot0 = np.zeros(NWINP + 1, np.int64)
    w_slot0[1:] = np.cumsum(S_per_w)

    order = np.lexsort((win, core))
    core_s, win_s = core[order], win[order]
    col_s, dstl_s, row_s = col[order], dstl[order], row[order]
    col2_s = (col_s // NLOC) * NLOC_PAD + (col_s % NLOC)
    invd_e = deg_inv[row_s]

    flat_cnt = cnt.reshape(-1)
    starts_flat = np.concatenate([[0], np.cumsum(flat_cnt)[:-1]])
    seg_starts = starts_flat.reshape(NC, NWINP)

    colA = np.zeros((NC, P, S_total), np.int32)
    colB = np.zeros((NC, P, S_total), np.int32)
    dstlA = np.full((NC, P, S_total), -1.0, np.float32)
    invdA = np.zeros((NC, P, S_total), np.float32)

    for c in range(NC):
        for w in range(NWINP):
            n = int(cnt[c, w])
            if n == 0:
                continue
            s0 = int(seg_starts[c, w])
            sl0 = int(w_slot0[w])
            k = np.arange(n)
            pp_, ss = k % P, sl0 + k // P
            colA[c, pp_, ss] = col_s[s0:s0 + n]
            colB[c, pp_, ss] = col2_s[s0:s0 + n]
            dstlA[c, pp_, ss] = dstl_s[s0:s0 + n]
            invdA[c, pp_, ss] = invd_e[s0:s0 + n]

    return colA, colB, dstlA, invdA, S_per_w, S_total


def _build(S_per_w, S_total, layer):
    """layer=1: x -> x1T.  layer=2: x1 -> outT (needs x1T input too)."""
    nc = bacc.Bacc("TRN2", target_bir_lowering=False, debug=False, num_devices=NC)
    nsrc = N if layer == 1 else NPAD
    src_d = nc.dram_tensor("src", [nsrc, D], f32, kind="ExternalInput")
    col_d = nc.dram_tensor("colx", [P, S_total], i32, kind="ExternalInput")
    dstl_d = nc.dram_tensor("dstl", [P, S_total], f32, kind="ExternalInput")
    invd_d = nc.dram_tensor("invd", [P, S_total], f32, kind="ExternalInput")
    W_d = nc.dram_tensor("W", [D, D], f32, kind="ExternalInput")
    b_d = nc.dram_tensor("b", [D, 1], f32, kind="ExternalInput")
    if layer == 2:
        x1T_d = nc.dram_tensor("x1T", [P, NLOC_PAD], f32, kind="ExternalInput")
        Wo1_d = nc.dram_tensor("Wo1", [D, D], f32, kind="ExternalInput")
        Wo2_d = nc.dram_tensor("Wo2", [D, D], f32, kind="ExternalInput")
        bo_d = nc.dram_tensor("bo", [D, 1], f32, kind="ExternalInput")
    outT_d = nc.dram_tensor("outT", [P, NLOC_PAD], f32, kind="ExternalOutput")

    w_slot0 = np.zeros(NWINP + 1, np.int64)
    w_slot0[1:] = np.cumsum(S_per_w)

    with tile.TileContext(nc) as tc:
        with tc.tile_pool(name="const", bufs=1) as cpool, \
             tc.tile_pool(name="meta", bufs=1) as mpool, \
             tc.tile_pool(name="work", bufs=8) as wpool, \
             tc.tile_pool(name="evict", bufs=3) as epool, \
             tc.tile_pool(name="psum", bufs=2, space="PSUM") as pp, \
             tc.tile_pool(name="psum2", bufs=2, space="PSUM") as pp2:

            def load_w(dram):
                t = cpool.tile([D, D], f32, tag=dram.name + "f")
                nc.sync.dma_start(out=t[:], in_=dram.ap())
                tr = cpool.tile([D, D], f32r, tag=dram.name + "r")
                nc.scalar.copy(out=tr[:], in_=t[:])
                return tr

            Wr = load_w(W_d)
            b_t = cpool.tile([D, 1], f32, tag="b")
            nc.sync.dma_start(out=b_t[:], in_=b_d.ap())
            if layer == 2:
                Wo1r, Wo2r = load_w(Wo1_d), load_w(Wo2_d)
                bo_t = cpool.tile([D, 1], f32, tag="bo")
                nc.sync.dma_start(out=bo_t[:], in_=bo_d.ap())
                x1T_t = mpool.tile([P, NLOC_PAD], f32)
                nc.sync.dma_start(out=x1T_t[:], in_=x1T_d.ap())

            iota_i = cpool.tile([P, WPAIR], i32, tag="ii")
            nc.gpsimd.iota(iota_i[:], pattern=[[1, WPAIR]], base=0, channel_multiplier=0)
            iota_f = cpool.tile([P, WPAIR], f32, tag="if")
            nc.vector.tensor_copy(out=iota_f[:], in_=iota_i[:])

            col_t = mpool.tile([P, S_total], i32)
            dstl_t = mpool.tile([P, S_total], f32)
            invd_t = mpool.tile([P, S_total], f32)
            nc.sync.dma_start(out=col_t[:], in_=col_d.ap())
            nc.sync.dma_start(out=dstl_t[:], in_=dstl_d.ap())
            nc.sync.dma_start(out=invd_t[:], in_=invd_d.ap())

            for w in range(NWINP):
                s0, s1 = int(w_slot0[w]), int(w_slot0[w + 1])
                o = w * WPAIR
                ps = pp.tile([D, WPAIR], f32, tag="agg", space="PSUM")
                for s in range(s0, s1):
                    xg = wpool.tile([P, D], f32, tag="xg")
                    nc.gpsimd.indirect_dma_start(
                        out=xg[:], out_offset=None, in_=src_d.ap(),
                        in_offset=bass.IndirectOffsetOnAxis(ap=col_t[:, s:s + 1], axis=0))
                    xgr = wpool.tile([P, D], f32r, tag="xgr")
                    nc.scalar.copy(out=xgr[:], in_=xg[:])
                    S = wpool.tile([P, WPAIR], f32r, tag="S")
                    nc.vector.tensor_scalar(
                        out=S[:], in0=iota_f[:],
                        scalar1=dstl_t[:, s:s + 1], scalar2=invd_t[:, s:s + 1],
                        op0=mybir.AluOpType.is_equal, op1=mybir.AluOpType.mult)
                    nc.tensor.matmul(out=ps[:], lhsT=xgr[:], rhs=S[:],
                                     start=(s == s0), stop=(s == s1 - 1))
                aggr = epool.tile([D, WPAIR], f32r, tag="aggr")
                nc.vector.tensor_copy(out=aggr[:], in_=ps[:])
                ps2 = pp2.tile([D, WPAIR], f32, tag="lin", space="PSUM")
                nc.tensor.matmul(out=ps2[:], lhsT=Wr[:], rhs=aggr[:], start=True, stop=True)
                if layer == 1:
                    hT = epool.tile([D, WPAIR], f32, tag="hT")
                    nc.scalar.activation(out=hT[:], in_=ps2[:],
                                         func=mybir.ActivationFunctionType.Relu,
                                         bias=b_t[:, :1], scale=1.0)
                    nc.sync.dma_start(out=outT_d.ap()[:, o:o + WPAIR], in_=hT[:])
                else:
                    x2Tr = epool.tile([D, WPAIR], f32r, tag="x2Tr")
                    nc.scalar.activation(out=x2Tr[:], in_=ps2[:],
                                         func=mybir.ActivationFunctionType.Relu,
                                         bias=b_t[:, :1], scale=1.0)
                    x1Tr = epool.tile([D, WPAIR], f32r, tag="x1Tr")
                    nc.scalar.copy(out=x1Tr[:], in_=x1T_t[:, o:o + WPAIR])
                    ps3 = pp2.tile([D, WPAIR], f32, tag="out", space="PSUM")
                    nc.tensor.matmul(out=ps3[:], lhsT=Wo1r[:], rhs=x1Tr[:], start=True, stop=False)
                    nc.tensor.matmul(out=ps3[:], lhsT=Wo2r[:], rhs=x2Tr[:], start=False, stop=True)
                    oT = epool.tile([D, WPAIR], f32, tag="oT")
                    nc.vector.tensor_scalar(
                        out=oT[:], in0=ps3[:], scalar1=bo_t[:, :1], scalar2=None,
                        op0=mybir.AluOpType.add)
                    nc.sync.dma_start(out=outT_d.ap()[:, o:o + WPAIR], in_=oT[:])

    nc.compile()
    return nc


def kernel(x, edge_index, W1, b1, W2, b2, Wout, bout):
    x = np.ascontiguousarray(np.asarray(x, np.float32))
    edge_index = np.asarray(edge_index, np.int32)

    deg = np.bincount(np.asarray(edge_index[0], np.int64), minlength=N).astype(np.float32) + 1.0
    deg_inv = (1.0 / deg).astype(np.float32)

    colA, colB, dstlA, invdA, S_per_w, S_total = _prep(edge_index, deg_inv)

    key = (S_total, tuple(S_per_w.tolist()))
    if _CACHE.get("key") != key:
        _CACHE.clear()
        _CACHE["key"] = key
        _CACHE[1] = _build(S_per_w, S_total, 1)
        _CACHE[2] = _build(S_per_w, S_total, 2)

    W1 = np.ascontiguousarray(np.asarray(W1, np.float32))
    W2 = np.ascontiguousarray(np.asarray(W2, np.float32))
    Wout = np.asarray(Wout, np.float32)
    b1 = np.asarray(b1, np.float32).reshape(D, 1)
    b2 = np.asarray(b2, np.float32).reshape(D, 1)
    bo = np.asarray(bout, np.float32).reshape(D, 1)

    in1 = [{"src": x, "colx": colA[c], "dstl": dstlA[c], "invd": invdA[c],
            "W": W1, "b": b1} for c in range(NC)]
    r1 = bass_utils.run_bass_kernel_spmd(_CACHE[1], in1, core_ids=list(range(NC)))
    x1T = np.stack([r1.results[c]["outT"] for c in range(NC)])   # [NC,128,NLOC_PAD]
    x1_full = np.ascontiguousarray(
        x1T.transpose(0, 2, 1).reshape(NPAD, D))                  # node-major padded

    in2 = [{"src": x1_full, "colx": colB[c], "dstl": dstlA[c], "invd": invdA[c],
            "W": W2, "b": b2, "x1T": x1T[c],
            "Wo1": np.ascontiguousarray(Wout[:D]),
            "Wo2": np.ascontiguousarray(Wout[D:]), "bo": bo} for c in range(NC)]
    r2 = bass_utils.run_bass_kernel_spmd(_CACHE[2], in2, core_ids=list(range(NC)))

    out = np.empty((N, D), np.float32)
    for c in range(NC):
        out[c * NLOC:(c + 1) * NLOC] = r2.results[c]["outT"].T[:NLOC]
    return out
